# revision 24
# baseline (speedup 1.0000x reference)
"""Trainium2 Bass kernel for modulated deformable conv v2 (DCNv2).

Problem (hardcoded): x [4,256,64,64] f32; offset_w [18,256,3,3]; offset_b [18];
mod_w [9,256,3,3]; mod_b [9]; weight [256,256,3,3] -> out [4,256,64,64] f32.

End-to-end latency is dominated by the ~43MB/s full-duplex axon pipe, so the
wire format is minimal and pipelined: the device module processes ONE full
image (both 32-row halves looped inside), weights/grids are device_put once
per call (~1.9MB), then 4 async jit calls stream one fp16 image each (2.1MB);
output fetches (fp16, 2.1MB/image) overlap later uploads via duplex. The
PJRT executable is cached across calls; identity matrices are NEFF consts.

Device algorithm per image, per half (r0 = 32*half):
  1. offset/mod conv (27 out ch) as accumulating matmuls with weights
     stationary (rhs = padded-x rows built on device from the image).
  2. index/weight math in [pos-partition, free] layout:
     py/px -> floor via round-to-nearest magic -> bilinear*2*sigmoid weights
     w00..w11 [128,NT,9] and int16 pixel indices into a 52x68(+2 guard)
     zero-ring padded table in half-slab coords (image rows r0-10..r0+41);
     indices in the gather's 16-row wrapped layout. The slab-relative grid
     is identical for both halves.
  3. per tap k: table y_k^T = x^T @ W_k^T on PE (fp16) for the in-image
     rows only; out-of-image rows and ring pads zero-DMA'd.
  4. per tap: 2 dma_gathers (rows y0, y0+1), payload = 2 adjacent pixels
     (512 fp16 = 1KB), landing [128 pos, NT, 512].
  5. combine: pos tiles 0..7 on DVE via scalar_tensor_tensor into f32 SBUF;
     pos tiles 8..15 on PE via scaled-identity diagonal matmuls accumulating
     in 4 PSUM banks; both drained to fp16 staging.
  6. DMA out [2048 pos, 256 o] fp16 per half; host restores NCHW f32.
"""

import numpy as np

B, C, H, W = 4, 256, 64, 64
O, K2 = 256, 9
ROWS = 32                  # output rows per half
P = ROWS * W               # positions per half = 2048
NT = P // 128              # position tiles per half = 16
TPW = 68                   # table row width in pixels
TROWS = 52                 # table rows: image rows r0-10 .. r0+41
TPIX = TROWS * TPW + 2     # +2 guard pixels = 3538
XR = ROWS + 2              # padded-x rows per half = 34
TQT = TROWS // 2           # table q-tiles (2 rows each) = 26

_CACHE = {}


def _patch_tile_drain():
    """This walrus build's TPB_CTRL encodes at most ~1 sem wait; Tile's
    kernel-tail drain aggregates the whole global clock onto one Drain.
    Spread the waits across a chain of single-wait drains instead."""
    import bass_rust
    from concourse.tile import TileContext, ScopedClock

    if getattr(TileContext, "_drain_patched", False):
        return

    def _drain_and_barrier(self, tick_clock, wait_clock):
        import os
        nc = self.nc
        drain_inst = nc.sync.drain()
        wait_clock.add_sem_waits(
            drain_inst.ins, ScopedClock({None: tick_clock.global_clock}))
        si = drain_inst.ins.sync_info
        if not os.environ.get("K_SIM") and si is not None \
                and len(si.on_wait) > 1:
            waits = list(si.on_wait)
            ups = list(si.on_update)
            drain_inst.ins.sync_info = bass_rust.SyncInfo(
                on_wait=waits[:1], on_update=ups)
            for j in range(1, len(waits)):
                extra = nc.sync.drain()
                extra.ins.sync_info = bass_rust.SyncInfo(
                    on_wait=[waits[j]], on_update=[])
        nc.all_engine_barrier()
        assert self.sems is not None
        popped = nc._tile_sem_poison_stack.pop()
        assert popped is self._sem_poison
        nc.clear_and_free_semaphores(list(self.sems.allocated().values()))
        nc.all_engine_barrier()

    TileContext._drain_and_barrier = _drain_and_barrier
    TileContext._drain_patched = True


def _build_module():
    import os
    import concourse.bass as bass
    import concourse.mybir as mybir
    import concourse.tile as tile
    from concourse.library_config import mlp as mlp_lib
    from contextlib import ExitStack

    _patch_tile_drain()

    dt = mybir.dt
    f32, f16, i16 = dt.float32, dt.float16, dt.int16
    Alu = mybir.AluOpType
    Act = mybir.ActivationFunctionType
    AP = bass.AP

    nc = bass.Bass(num_swdge_queues=4)

    x64_d = nc.dram_tensor("x64", [C, H * W], f16, kind="ExternalInput")
    wofs_d = nc.dram_tensor("wofs", [2, 128, 9, 27], f16, kind="ExternalInput")
    wtap_d = nc.dram_tensor("wtap", [2, 128, 9, O], f16, kind="ExternalInput")
    # bias[0, 0:18] = offset_b, bias[0, 18:27] = mod_b
    bias_d = nc.dram_tensor("bias", [1, 32], f32, kind="ExternalInput")
    id27_d = nc.inline_tensor(np.eye(27, dtype=np.float32), "id27")
    idn_d = nc.inline_tensor(np.eye(128, dtype=np.float16), "idn")
    # structural sampling grids in slab coords (identical for both halves,
    # bias added on device): bgyc[p,t,k] = row(p,t) + ky - 1 - 0.49999,
    # bgxc[p,t,k] = col(p) + kx - 1 - 0.49999 (magic-floor epsilon folded).
    _p = np.arange(P)
    _gy = np.empty((128, NT, 9), np.float32)
    _gx = np.empty((128, NT, 9), np.float32)
    for _k in range(9):
        _ky, _kx = divmod(_k, 3)
        _gy[:, :, _k] = (_p // 64 + 10 + _ky - 1 - 0.49999).reshape(NT, 128).T
        _gx[:, :, _k] = (_p % 64 + _kx - 1 - 0.49999).reshape(NT, 128).T
    bgyc_d = nc.inline_tensor(_gy.reshape(128, NT * 9), "bgyc")
    bgxc_d = nc.inline_tensor(_gx.reshape(128, NT * 9), "bgxc")
    out_d = nc.dram_tensor("out", [2 * P, O], f16, kind="ExternalOutput")

    # per-(half, tap) gather tables, double-buffered across halves
    tabs_d = [[nc.dram_tensor(f"tab{h}_{k}", [TPIX, O], f16)
               for k in range(K2)] for h in range(2)]

    with tile.TileContext(nc) as tc, ExitStack() as ctx:
        pool = ctx.enter_context(tc.tile_pool(name="main", bufs=1))
        psc = ctx.enter_context(tc.tile_pool(name="psc", bufs=1, space="PSUM"))
        pst = ctx.enter_context(tc.tile_pool(name="pst", bufs=3, space="PSUM"))
        pacc = ctx.enter_context(tc.tile_pool(name="pacc", bufs=1, space="PSUM"))
        dpool = ctx.enter_context(tc.tile_pool(name="diag", bufs=8))
        gpool = ctx.enter_context(tc.tile_pool(name="gath", bufs=2))
        spool = ctx.enter_context(tc.tile_pool(name="stage", bufs=6))

        # ---------------- load inputs ----------------
        nc.gpsimd.load_library(mlp_lib)
        x64 = pool.tile([128, 2, H * W], f16, tag="x64", name="x64_sb")
        nc.sync.dma_start(
            x64[:],
            AP(x64_d, 0, [[H * W, 128], [128 * H * W, 2], [1, H * W]]))
        wofs = pool.tile([128, 2, 9, 27], f16, tag="wofs", name="wofs_sb")
        nc.sync.dma_start(
            wofs[:],
            AP(wofs_d, 0, [[9 * 27, 128], [128 * 9 * 27, 2], [1, 9 * 27]]))
        wtap = pool.tile([128, 2, 9, O], f16, tag="wtap", name="wtap_sb")
        nc.sync.dma_start(
            wtap[:],
            AP(wtap_d, 0, [[9 * O, 128], [128 * 9 * O, 2], [1, 9 * O]]))
        bgy = pool.tile([128, NT, 9], f32, tag="bgy", name="bgy_sb")
        nc.sync.dma_start(bgy[:], bgyc_d[:, :])
        bgx = pool.tile([128, NT, 9], f32, tag="bgx", name="bgx_sb")
        nc.sync.dma_start(bgx[:], bgxc_d[:, :])
        # replicate the 32-float bias row across partitions (log doubling),
        # then fold the per-tap biases into the structural grids.
        bias = pool.tile([128, 32], f32, tag="bias", name="bias_sb")
        nc.sync.dma_start(bias[0:1, :], bias_d[:, :])
        for _d in range(7):
            w_ = 1 << _d
            nc.sync.dma_start(bias[w_:2 * w_, :], bias[0:w_, :])
        bias_y = AP(bias.tensor, bias.offset, [bias.ap[0], [0, NT], [2, 9]])
        bias_x = AP(bias.tensor, bias.offset + 1,
                    [bias.ap[0], [0, NT], [2, 9]])
        bias_m = AP(bias.tensor, bias.offset + 18,
                    [bias.ap[0], [0, NT], [1, 9]])
        nc.vector.tensor_tensor(bgy[:], bgy[:], bias_y, Alu.add)
        nc.vector.tensor_tensor(bgx[:], bgx[:], bias_x, Alu.add)
        id27 = pool.tile([27, 27], f32, tag="id27", name="id27_sb")
        nc.sync.dma_start(id27[:], id27_d[:, :])
        idn = pool.tile([128, 128], f16, tag="idn", name="idn_sb")
        nc.sync.dma_start(idn[:], idn_d[:, :])

        # zero tile for table zeroing (big band DMAs use all 1360 cols)
        zt = pool.tile([128, 1360], f16, tag="zt", name="zt_sb")
        nc.gpsimd.memset(zt[:], 0.0)

        # DVE-side combine accumulator (shared across halves, re-zeroed)
        accD = pool.tile([128, 8, O], f32, tag="accD", name="accD_sb")

        # one shared GPSIMD register for every gather's index count (72
        # per-call to_reg allocations would exhaust the register file)
        nidx_reg = nc.gpsimd.to_reg(P // 2)

        # ---------------- per-half pipeline ----------------
        for hf in range(2):
            r0 = ROWS * hf
            sfx = f"h{hf}"

            # -------- padded conv input [128c, 2ct, 34r x 66] --------
            # image rows r0-1 .. r0+32 into 66-wide zeroed rows at col 1.
            xpad = pool.tile([128, 2, XR * 66], f16, tag=f"xpad{sfx}",
                             name=f"xpad_{sfx}")
            nc.vector.memset(xpad[:], 0.0)
            a0, a1 = max(0, r0 - 1), min(H, r0 + XR - 1)
            for ct in range(2):
                xp_ct = xpad[:, ct, :]
                x6_ct = x64[:, ct, :]
                nc.sync.dma_start(
                    AP(xp_ct.tensor,
                       xp_ct.offset + (a0 - (r0 - 1)) * 66 + 1,
                       [xp_ct.ap[0], [66, a1 - a0], [1, W]]),
                    AP(x6_ct.tensor, x6_ct.offset + a0 * W,
                       [x6_ct.ap[0], [W, a1 - a0], [1, W]]))

            # -------- gather-table zeroing --------
            # valid slab rows: those with image rows r0-10+2qt in [0, 64);
            # h0: slab rows 10..51 valid (zero band rows 0..9);
            # h1: slab rows 0..41 valid (zero band rows 42..51).
            for k in range(K2):
                t = tabs_d[hf][k]
                if hf == 0:
                    # zero band px [0, 680) incl. its col pads
                    nc.scalar.dma_start(
                        AP(t, 0, [[1360, 128], [1, 1360]]), zt[:])
                    # col-pad runs (r,66),(r,67),(r+1,0),(r+1,1), r=10..50
                    nc.scalar.dma_start(
                        AP(t, (10 * 68 + 66) * O, [[68 * O, 41], [1, 4 * O]]),
                        zt[0:41, 0:1024])
                    # last-row right pads px 3534,3535
                    nc.sync.dma_start(
                        AP(t, (51 * 68 + 66) * O, [[1, 2 * O]]),
                        zt[0:1, 0:512])
                    # first-valid-row left pads px 680,681
                    nc.sync.dma_start(
                        AP(t, (10 * 68) * O, [[1, 2 * O]]), zt[0:1, 0:512])
                else:
                    # zero band px [2856, 3536)
                    nc.scalar.dma_start(
                        AP(t, 2856 * O, [[1360, 128], [1, 1360]]), zt[:])
                    # col-pad runs r=0..40
                    nc.scalar.dma_start(
                        AP(t, 66 * O, [[68 * O, 41], [1, 4 * O]]),
                        zt[0:41, 0:1024])
                    # row-0 left pads px 0,1
                    nc.sync.dma_start(AP(t, 0, [[1, 2 * O]]), zt[0:1, 0:512])
                    # last-valid-row right pads px 2854,2855
                    nc.sync.dma_start(
                        AP(t, (41 * 68 + 66) * O, [[1, 2 * O]]),
                        zt[0:1, 0:512])
                # guard px 3536,3537
                nc.sync.dma_start(
                    AP(t, (TROWS * 68) * O, [[1, 2 * O]]), zt[0:1, 0:512])

            # -------- offset/mod conv --------
            conv_sb = pool.tile([27, P], f32, tag=f"conv{sfx}",
                                name=f"conv_sb_{sfx}")
            for pc in range(4):
                ps = psc.tile([27, 512], f32, tag="convps",
                              name=f"convps_{sfx}_{pc}")
                n = 0
                for ct in range(2):
                    xp_ct = xpad[:, ct, :]
                    for tap in range(9):
                        dy, dx = divmod(tap, 3)
                        rhs = AP(xp_ct.tensor,
                                 xp_ct.offset + (8 * pc + dy) * 66 + dx,
                                 [xp_ct.ap[0], [66, 8], [1, 64]])
                        nc.tensor.matmul(
                            ps[:], wofs[:, ct, tap, :], rhs,
                            start=(n == 0), stop=(n == 17))
                        n += 1
                nc.scalar.activation(conv_sb[:, 512 * pc:512 * (pc + 1)],
                                     ps[:], Act.Copy)
            ofs = pool.tile([128, NT, 27], f32, tag=f"ofs{sfx}",
                            name=f"ofs_{sfx}")
            for pt in range(NT):
                ps2 = psc.tile([128, 27], f32, tag="convps",
                               name=f"trps_{sfx}_{pt}")
                nc.tensor.transpose(
                    ps2[:], conv_sb[:, 128 * pt:128 * (pt + 1)], id27[:])
                nc.scalar.activation(ofs[:, pt, :], ps2[:], Act.Copy)

            # -------- index/weight math --------
            def t144(nm):
                return pool.tile([128, NT, 9], f32, tag=f"{nm}{sfx}",
                                 name=f"{nm}_{sfx}")

            offy = AP(ofs.tensor, ofs.offset, [ofs.ap[0], [27, NT], [2, 9]])
            offx = AP(ofs.tensor, ofs.offset + 1,
                      [ofs.ap[0], [27, NT], [2, 9]])
            offm = AP(ofs.tensor, ofs.offset + 18,
                      [ofs.ap[0], [27, NT], [1, 9]])

            py, px = t144("py"), t144("px")
            nc.vector.tensor_tensor(py[:], offy, bgy[:], Alu.add)
            nc.vector.tensor_tensor(px[:], offx, bgx[:], Alu.add)

            # floor via round-to-nearest magic number: the host grids carry
            # -0.49999 so py here is py_true - 0.49999 and y0 = RN(py+M) - M
            # equals floor(py_true) (up to an O(1e-4) edge band, harmless).
            MAGIC = 12582912.0  # 1.5 * 2**23
            EPS = 0.49999
            fy, fx = t144("fy"), t144("fx")
            y0, x0 = t144("y0"), t144("x0")
            nc.vector.tensor_scalar(y0[:], py[:], MAGIC, -MAGIC,
                                    Alu.add, Alu.add)
            nc.vector.tensor_scalar(x0[:], px[:], MAGIC, -MAGIC,
                                    Alu.add, Alu.add)
            nc.vector.scalar_tensor_tensor(
                fy[:], py[:], EPS, y0[:], Alu.add, Alu.subtract)
            nc.vector.scalar_tensor_tensor(
                fx[:], px[:], EPS, x0[:], Alu.add, Alu.subtract)
            nc.vector.tensor_scalar(y0[:], y0[:], 0.0, float(TROWS - 2),
                                    Alu.max, Alu.min)
            nc.vector.tensor_scalar(x0[:], x0[:], -2.0, 64.0,
                                    Alu.max, Alu.min)

            # mask2 = 2*sigmoid(mod + mod_b); factor 2 folded into gy2/fy2
            m2 = t144("m2")
            nc.vector.tensor_tensor(m2[:], offm, bias_m, Alu.add)
            nc.scalar.activation(m2[:], m2[:], Act.Sigmoid)
            gy2, fy2 = t144("gy2"), t144("fy2")
            nc.vector.tensor_scalar(gy2[:], fy[:], -2.0, 2.0,
                                    Alu.mult, Alu.add)
            nc.vector.tensor_scalar(fy2[:], fy[:], 2.0, None, Alu.mult)
            gx1 = t144("gx1")
            nc.vector.tensor_scalar(gx1[:], fx[:], -1.0, 1.0,
                                    Alu.mult, Alu.add)
            wa, wb = t144("wa"), t144("wb")
            nc.vector.tensor_tensor(wa[:], gy2[:], m2[:], Alu.mult)
            nc.vector.tensor_tensor(wb[:], fy2[:], m2[:], Alu.mult)
            w00, w01 = t144("w00"), t144("w01")
            w10, w11 = t144("w10"), t144("w11")
            nc.vector.tensor_tensor(w00[:], wa[:], gx1[:], Alu.mult)
            nc.vector.tensor_tensor(w01[:], wa[:], fx[:], Alu.mult)
            nc.vector.tensor_tensor(w10[:], wb[:], gx1[:], Alu.mult)
            nc.vector.tensor_tensor(w11[:], wb[:], fx[:], Alu.mult)

            # indices, computed directly in the gather's wrapped layout:
            # partition r holds positions p = 16g + r; free = (k, i, t, g).
            ycS = pool.tile([16, 8, NT, 9], f32, tag=f"ycS{sfx}",
                            name=f"ycS_{sfx}")
            xcS = pool.tile([16, 8, NT, 9], f32, tag=f"xcS{sfx}",
                            name=f"xcS_{sfx}")
            for g in range(8):
                nc.sync.dma_start(ycS[0:16, g, :, :],
                                  y0[16 * g:16 * (g + 1), :, :])
                nc.sync.dma_start(xcS[0:16, g, :, :],
                                  x0[16 * g:16 * (g + 1), :, :])
            tfS = pool.tile([16, 8, NT, 9], f32, tag=f"tfS{sfx}",
                            name=f"tfS_{sfx}")
            nc.vector.scalar_tensor_tensor(
                tfS[:], ycS[:], 68.0, xcS[:], Alu.mult, Alu.add)
            i0S = pool.tile([16, 8, NT, 9], f32, tag=f"i0S{sfx}",
                            name=f"i0S_{sfx}")
            i1S = pool.tile([16, 8, NT, 9], f32, tag=f"i1S{sfx}",
                            name=f"i1S_{sfx}")
            nc.vector.tensor_scalar(i0S[:], tfS[:], 2.0, None, Alu.add)
            nc.vector.tensor_scalar(i1S[:], tfS[:], 70.0, None, Alu.add)
            idxR = pool.tile([128, 9, 2, NT, 8], i16, tag=f"idxR{sfx}",
                             name=f"idxR_{sfx}")
            for i, iS in ((0, i0S), (1, i1S)):
                out_ap = AP(idxR.tensor, idxR.offset + i * 128,
                            [[idxR.ap[0][0], 16], [1, 8], [8, NT], [256, 9]])
                nc.vector.tensor_copy(out_ap, iS[:])
            for cg in range(1, 8):
                nc.sync.dma_start(
                    idxR[16 * cg:16 * (cg + 1), :, :, :, :],
                    idxR[0:16, :, :, :, :])

            # re-zero the shared DVE accumulator for this half
            nc.vector.memset(accD[:], 0.0)
            # fp16 output staging for this half
            accH = pool.tile([128, NT, O], f16, tag=f"accH{sfx}",
                             name=f"accH_{sfx}")

            # psum accumulators for the PE-side combine (pos tiles 8..15)
            pa = [pacc.tile([128, 2, O], f32, tag=f"pa{j}",
                            name=f"pa_{sfx}_{j}")
                  for j in range(4)]

            # -------- per-tap: table, gather, combine --------
            qt_lo = 5 if hf == 0 else 0            # valid slab q-tiles
            qt_hi = 26 if hf == 0 else 21
            for k in range(K2):
                qts = list(range(qt_lo, qt_hi))
                pairs = [qts[j:j + 2] for j in range(0, len(qts), 2)]
                for pi, pr in enumerate(pairs):
                    ps = pst.tile([128, 2, O], f32, tag="tabps",
                                  name=f"tabps_{sfx}_{k}_{pi}")
                    for h, qt in enumerate(pr):
                        ir = r0 - 10 + 2 * qt      # image row of slab 2qt
                        for ct in range(2):
                            nc.tensor.matmul(
                                ps[:, h, :],
                                x64[:, ct, ir * W:(ir + 2) * W],
                                wtap[:, ct, k, :],
                                start=(ct == 0), stop=(ct == 1))
                    st = spool.tile([128, 2, O], f16, tag="tabst",
                                    name=f"tabst_{sfx}_{k}_{pi}")
                    nc.scalar.activation(st[:, 0:len(pr), :],
                                         ps[:, 0:len(pr), :], Act.Copy)
                    for h, qt in enumerate(pr):
                        # spread table-write DMAs over the HWDGE rings
                        weng = (nc.sync, nc.scalar)[(13 * k + pi) % 2]
                        weng.dma_start(
                            AP(tabs_d[hf][k], (2 * qt * 68 + 2) * O,
                               [[68 * O, 2], [O, 64], [1, O]]),
                            st[:, h, :])
                for i in range(2):
                    G = gpool.tile([128, NT, 512], f16, tag="G",
                                   name=f"G_{sfx}_{k}_{i}")
                    tab_ap = AP(tabs_d[hf][k], 0, [[O, TPIX - 1], [1, 512]])
                    # two half-gathers: idx<1024 covers pos tiles 0..7 (DVE
                    # combine half), idx>=1024 tiles 8..15 (PE half)
                    for hh in range(2):
                        nc.gpsimd.dma_gather(
                            G[:, 8 * hh:8 * (hh + 1), :], tab_ap,
                            idxR[:, k, i, 8 * hh:8 * (hh + 1), :],
                            num_idxs=P // 2, num_idxs_reg=nidx_reg,
                            elem_size=512, elem_step=O,
                            queue_num=(4 * k + 2 * i + hh) % 4,
                            single_packet=False)
                    wlo = w00 if i == 0 else w10
                    whi = w01 if i == 0 else w11
                    for pt in range(8):
                        nc.vector.scalar_tensor_tensor(
                            accD[:, pt, :], G[:, pt, 0:O],
                            wlo[:, pt, k:k + 1],
                            accD[:, pt, :], Alu.mult, Alu.add)
                        nc.vector.scalar_tensor_tensor(
                            accD[:, pt, :], G[:, pt, O:2 * O],
                            whi[:, pt, k:k + 1],
                            accD[:, pt, :], Alu.mult, Alu.add)
                    # pos tiles 8..15: scaled-identity matmuls accum in PSUM
                    for pt in range(8, NT):
                        for pix, wv in ((0, wlo), (1, whi)):
                            t = 4 * k + 2 * i + pix
                            dg = dpool.tile([128, 128], f16, tag="dg",
                                            name=f"dg_{sfx}_{k}_{i}_{pt}_{pix}")
                            if t % 3 == 0:
                                nc.vector.tensor_scalar(
                                    dg[:], idn[:], wv[:, pt, k:k + 1], None,
                                    Alu.mult)
                            else:
                                nc.scalar.activation(
                                    dg[:], idn[:], Act.Copy,
                                    scale=wv[:, pt, k:k + 1])
                            pb = pa[(pt - 8) // 2]
                            nc.tensor.matmul(
                                pb[:, (pt - 8) % 2, :], dg[:],
                                G[:, pt, pix * O:(pix + 1) * O],
                                start=(t == 0 and (pt - 8) % 2 == 0),
                                stop=(t == 35 and (pt - 8) % 2 == 1),
                                skip_group_check=True)

            # drain PE-side psum accumulators straight to fp16 staging
            for pt in range(8, NT):
                nc.scalar.activation(
                    accH[:, pt, :], pa[(pt - 8) // 2][:, (pt - 8) % 2, :],
                    Act.Copy)
            # DVE-side f32 accumulators -> fp16
            nc.scalar.activation(accH[:, 0:8, :], accD[:], Act.Copy)

            # -------- output --------
            base = hf * P * O
            nc.sync.dma_start(
                AP(out_d, base, [[O, 128], [128 * O, 8], [1, O]]),
                accH[:, 0:8, :])
            nc.sync.dma_start(
                AP(out_d, base + 8 * 128 * O, [[O, 128], [128 * O, 8], [1, O]]),
                accH[:, 8:NT, :])

    from concourse.library_overlay import lower_extended_insts
    lower_extended_insts(nc)
    if not os.environ.get("K_SIM"):
        _split_sync_waits(nc)
    return nc


def _split_sync_waits(nc, max_waits=1):
    """This walrus build encodes at most ~1 sem wait per instruction.
    Hoist extra waits onto preceding same-engine EventSemaphore ops."""
    import bass_rust
    import concourse.mybir as mybir
    for f in nc.m.functions:
        for bb in f.blocks:
            out = []
            changed = False
            for ins in bb.instructions:
                si = ins.sync_info
                if si is not None and len(si.on_wait) > max_waits \
                        and ins.engine is not None:
                    waits = list(si.on_wait)
                    extras, keep = waits[:-max_waits], waits[-max_waits:]
                    for j in range(0, len(extras), max_waits):
                        evs = mybir.InstNoOp(
                            name=f"nop_split_{nc.next_id()}", ins=[], outs=[],
                            engine=ins.engine)
                        evs.sync_info = bass_rust.SyncInfo(
                            on_wait=extras[j:j + max_waits], on_update=[])
                        out.append(evs)
                    ins.sync_info = bass_rust.SyncInfo(
                        on_wait=keep, on_update=list(si.on_update))
                    changed = True
                out.append(ins)
            if changed:
                bb.instructions = out


def _prep_host(inputs):
    """Build per-image x and the shared weight/grid arrays."""
    x = np.asarray(inputs["x"], np.float32)
    offset_w = np.asarray(inputs["offset_w"], np.float32)
    offset_b = np.asarray(inputs["offset_b"], np.float32)
    mod_w = np.asarray(inputs["mod_w"], np.float32)
    mod_b = np.asarray(inputs["mod_b"], np.float32)
    weight = np.asarray(inputs["weight"], np.float32)

    x64 = x.reshape(B, C, H * W)   # cast to fp16 per image at upload time

    wofs = np.concatenate([offset_w, mod_w], 0)            # [27, C, 3, 3]
    wofs = wofs.transpose(2, 3, 1, 0).reshape(9, C, 27)    # [tap, c, 27]
    wofs = np.ascontiguousarray(
        wofs.transpose(1, 0, 2).reshape(2, 128, 9, 27)).astype(np.float16)

    wtap = weight.reshape(O, C, 9).transpose(2, 1, 0)      # [tap, c, o]
    wtap = np.ascontiguousarray(
        wtap.transpose(1, 0, 2).reshape(2, 128, 9, O)).astype(np.float16)

    bias = np.zeros((1, 32), np.float32)
    bias[0, 0:18] = offset_b
    bias[0, 18:27] = mod_b

    return {
        "x64": x64,
        "wofs": wofs,
        "wtap": wtap,
        "bias": bias,
    }


def _get_runner():
    """Build the bass module once and wrap it in a cached jit fn."""
    if "runner" in _CACHE:
        return _CACHE["runner"]

    import sys
    if "/opt/trn_rl_repo" not in sys.path:
        sys.path.insert(0, "/opt/trn_rl_repo")
    import jax
    import concourse.mybir as mybir
    from concourse.bass2jax import (
        install_neuronx_cc_hook, _bass_exec_p, partition_id_tensor)

    nc = _build_module()
    install_neuronx_cc_hook()

    partition_name = (nc.partition_id_tensor.name
                      if nc.partition_id_tensor else None)
    in_names, out_names, out_avals = [], [], []
    for alloc in nc.m.functions[0].allocations:
        if not isinstance(alloc, mybir.MemoryLocationSet):
            continue
        name = alloc.memorylocations[0].name
        if alloc.kind == "ExternalInput":
            if name != partition_name:
                in_names.append(name)
        elif alloc.kind == "ExternalOutput":
            out_avals.append(jax.core.ShapedArray(
                tuple(alloc.tensor_shape), mybir.dt.np(alloc.dtype)))
            out_names.append(name)
    all_names = list(in_names)
    if partition_name is not None:
        all_names.append(partition_name)

    def _body(*args):
        # NOTE: no output-buffer operands — the PJRT runtime allocates NEFF
        # outputs itself and the kernel fully overwrites "out". The
        # neuronx_cc hook requires every operand to be a plain parameter.
        operands = list(args)
        if partition_name is not None:
            operands.append(partition_id_tensor())
        outs = _bass_exec_p.bind(
            *operands, out_avals=tuple(out_avals), in_names=tuple(all_names),
            out_names=tuple(out_names), lowering_input_output_aliases=(),
            sim_require_finite=True, sim_require_nnan=True, nc=nc)
        return tuple(outs)

    jit_fn = jax.jit(_body)
    _CACHE["runner"] = (jit_fn, in_names)
    return _CACHE["runner"]


def kernel(trace=False, **inputs):
    import jax
    from concurrent.futures import ThreadPoolExecutor
    jit_fn, in_names = _get_runner()
    feed = _prep_host(inputs)
    x32 = feed["x64"]
    # Two devices = two independent FIFO queues: device 1's uploads overlap
    # device 0's output downloads (the axon pipe is full-duplex but each
    # device executes its queue strictly in order). Weights/grids ship once
    # per device; per-image x is cast right before its async upload, and
    # each call is dispatched before the next upload is enqueued so image
    # b's exec isn't queued behind image b+1's transfer. Fetches run in
    # four threads so each output's download starts the moment its exec
    # finishes.
    devs = jax.devices()

    def submit(d_i):
        dev = devs[d_i]
        shared = {n: jax.device_put(feed[n], dev)
                  for n in in_names if n != "x64"}
        outs, first_x = [], None
        for b in (2 * d_i, 2 * d_i + 1):
            xb = jax.device_put(x32[b].astype(np.float16), dev)
            if first_x is None:
                first_x = xb
            args = [xb if n == "x64" else shared[n] for n in in_names]
            o = jit_fn(*args)[0]
            # start the output download server-side the moment its exec
            # finishes — the later np.asarray is then (nearly) free and no
            # per-fetch ~80ms client round trip lands on the critical path.
            o.copy_to_host_async()
            outs.append(o)
        return outs, first_x

    outs01, x0_dev = submit(0)
    # gate device 1's submission on image 0's upload: concurrent uploads
    # fair-share the pipe, which would delay every exec (and the duplexed
    # downloads) to the very end of the combined upload stream.
    outs23 = []
    import threading

    def submit_dev1():
        x0_dev.block_until_ready()
        outs23.extend(submit(1)[0])

    th = threading.Thread(target=submit_dev1)
    th.start()
    res = [np.asarray(o) for o in outs01]
    th.join()
    res += [np.asarray(o) for o in outs23]
    out = np.empty((B, O, H, W), np.float32)
    for b in range(B):
        out[b] = res[b].reshape(2, ROWS, W, O).transpose(
            3, 0, 1, 2).reshape(O, H, W)
    return out


# revision 26
# speedup vs baseline: 1.0362x; 1.0362x over previous
"""Trainium2 Bass kernel for modulated deformable conv v2 (DCNv2).

Problem (hardcoded): x [4,256,64,64] f32; offset_w [18,256,3,3]; offset_b [18];
mod_w [9,256,3,3]; mod_b [9]; weight [256,256,3,3] -> out [4,256,64,64] f32.

End-to-end latency is dominated by the ~43MB/s full-duplex axon pipe, so the
wire format is minimal and pipelined: the device module processes ONE full
image (both 32-row halves looped inside), weights/grids are device_put once
per call (~1.9MB), then 4 async jit calls stream one fp16 image each (2.1MB);
output fetches (fp16, 2.1MB/image) overlap later uploads via duplex. The
PJRT executable is cached across calls; identity matrices are NEFF consts.

Device algorithm per image, per half (r0 = 32*half):
  1. offset/mod conv (27 out ch) as accumulating matmuls with weights
     stationary (rhs = padded-x rows built on device from the image).
  2. index/weight math in [pos-partition, free] layout:
     py/px -> floor via round-to-nearest magic -> bilinear*2*sigmoid weights
     w00..w11 [128,NT,9] and int16 pixel indices into a 52x68(+2 guard)
     zero-ring padded table in half-slab coords (image rows r0-10..r0+41);
     indices in the gather's 16-row wrapped layout. The slab-relative grid
     is identical for both halves.
  3. per tap k: table y_k^T = x^T @ W_k^T on PE (fp16) for the in-image
     rows only; out-of-image rows and ring pads zero-DMA'd.
  4. per tap: 2 dma_gathers (rows y0, y0+1), payload = 2 adjacent pixels
     (512 fp16 = 1KB), landing [128 pos, NT, 512].
  5. combine: pos tiles 0..7 on DVE via scalar_tensor_tensor into f32 SBUF;
     pos tiles 8..15 on PE via scaled-identity diagonal matmuls accumulating
     in 4 PSUM banks; both drained to fp16 staging.
  6. DMA out [2048 pos, 256 o] fp16 per half; host restores NCHW f32.
"""

import numpy as np

B, C, H, W = 4, 256, 64, 64
O, K2 = 256, 9
ROWS = 32                  # output rows per half
P = ROWS * W               # positions per half = 2048
NT = P // 128              # position tiles per half = 16
TPW = 68                   # table row width in pixels
TROWS = 52                 # table rows: image rows r0-10 .. r0+41
TPIX = TROWS * TPW + 2     # +2 guard pixels = 3538
XR = ROWS + 2              # padded-x rows per half = 34
TQT = TROWS // 2           # table q-tiles (2 rows each) = 26

_CACHE = {}


def _patch_tile_drain():
    """This walrus build's TPB_CTRL encodes at most ~1 sem wait; Tile's
    kernel-tail drain aggregates the whole global clock onto one Drain.
    Spread the waits across a chain of single-wait drains instead."""
    import bass_rust
    from concourse.tile import TileContext, ScopedClock

    if getattr(TileContext, "_drain_patched", False):
        return

    def _drain_and_barrier(self, tick_clock, wait_clock):
        import os
        nc = self.nc
        drain_inst = nc.sync.drain()
        wait_clock.add_sem_waits(
            drain_inst.ins, ScopedClock({None: tick_clock.global_clock}))
        si = drain_inst.ins.sync_info
        if not os.environ.get("K_SIM") and si is not None \
                and len(si.on_wait) > 1:
            waits = list(si.on_wait)
            ups = list(si.on_update)
            drain_inst.ins.sync_info = bass_rust.SyncInfo(
                on_wait=waits[:1], on_update=ups)
            for j in range(1, len(waits)):
                extra = nc.sync.drain()
                extra.ins.sync_info = bass_rust.SyncInfo(
                    on_wait=[waits[j]], on_update=[])
        nc.all_engine_barrier()
        assert self.sems is not None
        popped = nc._tile_sem_poison_stack.pop()
        assert popped is self._sem_poison
        nc.clear_and_free_semaphores(list(self.sems.allocated().values()))
        nc.all_engine_barrier()

    TileContext._drain_and_barrier = _drain_and_barrier
    TileContext._drain_patched = True


def _build_module():
    import os
    import concourse.bass as bass
    import concourse.mybir as mybir
    import concourse.tile as tile
    from concourse.library_config import mlp as mlp_lib
    from contextlib import ExitStack

    _patch_tile_drain()

    dt = mybir.dt
    f32, f16, i16 = dt.float32, dt.float16, dt.int16
    Alu = mybir.AluOpType
    Act = mybir.ActivationFunctionType
    AP = bass.AP

    nc = bass.Bass(num_swdge_queues=4)

    x64_d = nc.dram_tensor("x64", [C, H * W], f16, kind="ExternalInput")
    wofs_d = nc.dram_tensor("wofs", [2, 128, 9, 27], f16, kind="ExternalInput")
    wtap_d = nc.dram_tensor("wtap", [2, 128, 9, O], f16, kind="ExternalInput")
    # bias[0, 0:18] = offset_b, bias[0, 18:27] = mod_b
    bias_d = nc.dram_tensor("bias", [1, 32], f32, kind="ExternalInput")
    id27_d = nc.inline_tensor(np.eye(27, dtype=np.float32), "id27")
    idn_d = nc.inline_tensor(np.eye(128, dtype=np.float16), "idn")
    # structural sampling grids in slab coords (identical for both halves,
    # bias added on device): bgyc[p,t,k] = row(p,t) + ky - 1 - 0.49999,
    # bgxc[p,t,k] = col(p) + kx - 1 - 0.49999 (magic-floor epsilon folded).
    _p = np.arange(P)
    _gy = np.empty((128, NT, 9), np.float32)
    _gx = np.empty((128, NT, 9), np.float32)
    for _k in range(9):
        _ky, _kx = divmod(_k, 3)
        _gy[:, :, _k] = (_p // 64 + 10 + _ky - 1 - 0.49999).reshape(NT, 128).T
        _gx[:, :, _k] = (_p % 64 + _kx - 1 - 0.49999).reshape(NT, 128).T
    bgyc_d = nc.inline_tensor(_gy.reshape(128, NT * 9), "bgyc")
    bgxc_d = nc.inline_tensor(_gx.reshape(128, NT * 9), "bgxc")
    out_d = nc.dram_tensor("out", [2 * P, O], f16, kind="ExternalOutput")

    # per-(half, tap) gather tables, double-buffered across halves
    tabs_d = [[nc.dram_tensor(f"tab{h}_{k}", [TPIX, O], f16)
               for k in range(K2)] for h in range(2)]

    with tile.TileContext(nc) as tc, ExitStack() as ctx:
        pool = ctx.enter_context(tc.tile_pool(name="main", bufs=1))
        psc = ctx.enter_context(tc.tile_pool(name="psc", bufs=1, space="PSUM"))
        pst = ctx.enter_context(tc.tile_pool(name="pst", bufs=3, space="PSUM"))
        pacc = ctx.enter_context(tc.tile_pool(name="pacc", bufs=1, space="PSUM"))
        dpool = ctx.enter_context(tc.tile_pool(name="diag", bufs=8))
        gpool = ctx.enter_context(tc.tile_pool(name="gath", bufs=2))
        spool = ctx.enter_context(tc.tile_pool(name="stage", bufs=6))

        # ---------------- load inputs ----------------
        nc.gpsimd.load_library(mlp_lib)
        x64 = pool.tile([128, 2, H * W], f16, tag="x64", name="x64_sb")
        nc.sync.dma_start(
            x64[:],
            AP(x64_d, 0, [[H * W, 128], [128 * H * W, 2], [1, H * W]]))
        wofs = pool.tile([128, 2, 9, 27], f16, tag="wofs", name="wofs_sb")
        nc.sync.dma_start(
            wofs[:],
            AP(wofs_d, 0, [[9 * 27, 128], [128 * 9 * 27, 2], [1, 9 * 27]]))
        wtap = pool.tile([128, 2, 9, O], f16, tag="wtap", name="wtap_sb")
        nc.sync.dma_start(
            wtap[:],
            AP(wtap_d, 0, [[9 * O, 128], [128 * 9 * O, 2], [1, 9 * O]]))
        bgy = pool.tile([128, NT, 9], f32, tag="bgy", name="bgy_sb")
        nc.sync.dma_start(bgy[:], bgyc_d[:, :])
        bgx = pool.tile([128, NT, 9], f32, tag="bgx", name="bgx_sb")
        nc.sync.dma_start(bgx[:], bgxc_d[:, :])
        # replicate the 32-float bias row across partitions (log doubling),
        # then fold the per-tap biases into the structural grids.
        bias = pool.tile([128, 32], f32, tag="bias", name="bias_sb")
        nc.sync.dma_start(bias[0:1, :], bias_d[:, :])
        for _d in range(7):
            w_ = 1 << _d
            nc.sync.dma_start(bias[w_:2 * w_, :], bias[0:w_, :])
        bias_y = AP(bias.tensor, bias.offset, [bias.ap[0], [0, NT], [2, 9]])
        bias_x = AP(bias.tensor, bias.offset + 1,
                    [bias.ap[0], [0, NT], [2, 9]])
        bias_m = AP(bias.tensor, bias.offset + 18,
                    [bias.ap[0], [0, NT], [1, 9]])
        nc.vector.tensor_tensor(bgy[:], bgy[:], bias_y, Alu.add)
        nc.vector.tensor_tensor(bgx[:], bgx[:], bias_x, Alu.add)
        id27 = pool.tile([27, 27], f32, tag="id27", name="id27_sb")
        nc.sync.dma_start(id27[:], id27_d[:, :])
        idn = pool.tile([128, 128], f16, tag="idn", name="idn_sb")
        nc.sync.dma_start(idn[:], idn_d[:, :])

        # zero tile for table zeroing (big band DMAs use all 1360 cols)
        zt = pool.tile([128, 1360], f16, tag="zt", name="zt_sb")
        nc.gpsimd.memset(zt[:], 0.0)

        # DVE-side combine accumulator (shared across halves, re-zeroed)
        accD = pool.tile([128, 8, O], f32, tag="accD", name="accD_sb")

        # one shared GPSIMD register for every gather's index count (72
        # per-call to_reg allocations would exhaust the register file)
        nidx_reg = nc.gpsimd.to_reg(P // 2)

        # ---------------- per-half pipeline ----------------
        for hf in range(2):
            r0 = ROWS * hf
            sfx = f"h{hf}"

            # -------- padded conv input [128c, 2ct, 34r x 66] --------
            # image rows r0-1 .. r0+32 into 66-wide zeroed rows at col 1.
            xpad = pool.tile([128, 2, XR * 66], f16, tag=f"xpad{sfx}",
                             name=f"xpad_{sfx}")
            nc.vector.memset(xpad[:], 0.0)
            a0, a1 = max(0, r0 - 1), min(H, r0 + XR - 1)
            for ct in range(2):
                xp_ct = xpad[:, ct, :]
                x6_ct = x64[:, ct, :]
                nc.sync.dma_start(
                    AP(xp_ct.tensor,
                       xp_ct.offset + (a0 - (r0 - 1)) * 66 + 1,
                       [xp_ct.ap[0], [66, a1 - a0], [1, W]]),
                    AP(x6_ct.tensor, x6_ct.offset + a0 * W,
                       [x6_ct.ap[0], [W, a1 - a0], [1, W]]))

            # -------- gather-table zeroing --------
            # valid slab rows: those with image rows r0-10+2qt in [0, 64);
            # h0: slab rows 10..51 valid (zero band rows 0..9);
            # h1: slab rows 0..41 valid (zero band rows 42..51).
            for k in range(K2):
                t = tabs_d[hf][k]
                if hf == 0:
                    # zero band px [0, 680) incl. its col pads
                    nc.scalar.dma_start(
                        AP(t, 0, [[1360, 128], [1, 1360]]), zt[:])
                    # col-pad runs (r,66),(r,67),(r+1,0),(r+1,1), r=10..50
                    nc.scalar.dma_start(
                        AP(t, (10 * 68 + 66) * O, [[68 * O, 41], [1, 4 * O]]),
                        zt[0:41, 0:1024])
                    # last-row right pads px 3534,3535
                    nc.sync.dma_start(
                        AP(t, (51 * 68 + 66) * O, [[1, 2 * O]]),
                        zt[0:1, 0:512])
                    # first-valid-row left pads px 680,681
                    nc.sync.dma_start(
                        AP(t, (10 * 68) * O, [[1, 2 * O]]), zt[0:1, 0:512])
                else:
                    # zero band px [2856, 3536)
                    nc.scalar.dma_start(
                        AP(t, 2856 * O, [[1360, 128], [1, 1360]]), zt[:])
                    # col-pad runs r=0..40
                    nc.scalar.dma_start(
                        AP(t, 66 * O, [[68 * O, 41], [1, 4 * O]]),
                        zt[0:41, 0:1024])
                    # row-0 left pads px 0,1
                    nc.sync.dma_start(AP(t, 0, [[1, 2 * O]]), zt[0:1, 0:512])
                    # last-valid-row right pads px 2854,2855
                    nc.sync.dma_start(
                        AP(t, (41 * 68 + 66) * O, [[1, 2 * O]]),
                        zt[0:1, 0:512])
                # guard px 3536,3537
                nc.sync.dma_start(
                    AP(t, (TROWS * 68) * O, [[1, 2 * O]]), zt[0:1, 0:512])

            # -------- offset/mod conv --------
            conv_sb = pool.tile([27, P], f32, tag=f"conv{sfx}",
                                name=f"conv_sb_{sfx}")
            for pc in range(4):
                ps = psc.tile([27, 512], f32, tag="convps",
                              name=f"convps_{sfx}_{pc}")
                n = 0
                for ct in range(2):
                    xp_ct = xpad[:, ct, :]
                    for tap in range(9):
                        dy, dx = divmod(tap, 3)
                        rhs = AP(xp_ct.tensor,
                                 xp_ct.offset + (8 * pc + dy) * 66 + dx,
                                 [xp_ct.ap[0], [66, 8], [1, 64]])
                        nc.tensor.matmul(
                            ps[:], wofs[:, ct, tap, :], rhs,
                            start=(n == 0), stop=(n == 17))
                        n += 1
                nc.scalar.activation(conv_sb[:, 512 * pc:512 * (pc + 1)],
                                     ps[:], Act.Copy)
            ofs = pool.tile([128, NT, 27], f32, tag=f"ofs{sfx}",
                            name=f"ofs_{sfx}")
            for pt in range(NT):
                ps2 = psc.tile([128, 27], f32, tag="convps",
                               name=f"trps_{sfx}_{pt}")
                nc.tensor.transpose(
                    ps2[:], conv_sb[:, 128 * pt:128 * (pt + 1)], id27[:])
                nc.scalar.activation(ofs[:, pt, :], ps2[:], Act.Copy)

            # -------- index/weight math --------
            def t144(nm):
                return pool.tile([128, NT, 9], f32, tag=f"{nm}{sfx}",
                                 name=f"{nm}_{sfx}")

            offy = AP(ofs.tensor, ofs.offset, [ofs.ap[0], [27, NT], [2, 9]])
            offx = AP(ofs.tensor, ofs.offset + 1,
                      [ofs.ap[0], [27, NT], [2, 9]])
            offm = AP(ofs.tensor, ofs.offset + 18,
                      [ofs.ap[0], [27, NT], [1, 9]])

            py, px = t144("py"), t144("px")
            nc.vector.tensor_tensor(py[:], offy, bgy[:], Alu.add)
            nc.vector.tensor_tensor(px[:], offx, bgx[:], Alu.add)

            # floor via round-to-nearest magic number: the host grids carry
            # -0.49999 so py here is py_true - 0.49999 and y0 = RN(py+M) - M
            # equals floor(py_true) (up to an O(1e-4) edge band, harmless).
            MAGIC = 12582912.0  # 1.5 * 2**23
            EPS = 0.49999
            fy, fx = t144("fy"), t144("fx")
            y0, x0 = t144("y0"), t144("x0")
            nc.vector.tensor_scalar(y0[:], py[:], MAGIC, -MAGIC,
                                    Alu.add, Alu.add)
            nc.vector.tensor_scalar(x0[:], px[:], MAGIC, -MAGIC,
                                    Alu.add, Alu.add)
            nc.vector.scalar_tensor_tensor(
                fy[:], py[:], EPS, y0[:], Alu.add, Alu.subtract)
            nc.vector.scalar_tensor_tensor(
                fx[:], px[:], EPS, x0[:], Alu.add, Alu.subtract)
            nc.vector.tensor_scalar(y0[:], y0[:], 0.0, float(TROWS - 2),
                                    Alu.max, Alu.min)
            nc.vector.tensor_scalar(x0[:], x0[:], -2.0, 64.0,
                                    Alu.max, Alu.min)

            # mask2 = 2*sigmoid(mod + mod_b); factor 2 folded into gy2/fy2
            m2 = t144("m2")
            nc.vector.tensor_tensor(m2[:], offm, bias_m, Alu.add)
            nc.scalar.activation(m2[:], m2[:], Act.Sigmoid)
            gy2, fy2 = t144("gy2"), t144("fy2")
            nc.vector.tensor_scalar(gy2[:], fy[:], -2.0, 2.0,
                                    Alu.mult, Alu.add)
            nc.vector.tensor_scalar(fy2[:], fy[:], 2.0, None, Alu.mult)
            gx1 = t144("gx1")
            nc.vector.tensor_scalar(gx1[:], fx[:], -1.0, 1.0,
                                    Alu.mult, Alu.add)
            wa, wb = t144("wa"), t144("wb")
            nc.vector.tensor_tensor(wa[:], gy2[:], m2[:], Alu.mult)
            nc.vector.tensor_tensor(wb[:], fy2[:], m2[:], Alu.mult)
            w00, w01 = t144("w00"), t144("w01")
            w10, w11 = t144("w10"), t144("w11")
            nc.vector.tensor_tensor(w00[:], wa[:], gx1[:], Alu.mult)
            nc.vector.tensor_tensor(w01[:], wa[:], fx[:], Alu.mult)
            nc.vector.tensor_tensor(w10[:], wb[:], gx1[:], Alu.mult)
            nc.vector.tensor_tensor(w11[:], wb[:], fx[:], Alu.mult)

            # indices, computed directly in the gather's wrapped layout:
            # partition r holds positions p = 16g + r; free = (k, i, t, g).
            ycS = pool.tile([16, 8, NT, 9], f32, tag=f"ycS{sfx}",
                            name=f"ycS_{sfx}")
            xcS = pool.tile([16, 8, NT, 9], f32, tag=f"xcS{sfx}",
                            name=f"xcS_{sfx}")
            for g in range(8):
                nc.sync.dma_start(ycS[0:16, g, :, :],
                                  y0[16 * g:16 * (g + 1), :, :])
                nc.sync.dma_start(xcS[0:16, g, :, :],
                                  x0[16 * g:16 * (g + 1), :, :])
            tfS = pool.tile([16, 8, NT, 9], f32, tag=f"tfS{sfx}",
                            name=f"tfS_{sfx}")
            nc.vector.scalar_tensor_tensor(
                tfS[:], ycS[:], 68.0, xcS[:], Alu.mult, Alu.add)
            i0S = pool.tile([16, 8, NT, 9], f32, tag=f"i0S{sfx}",
                            name=f"i0S_{sfx}")
            i1S = pool.tile([16, 8, NT, 9], f32, tag=f"i1S{sfx}",
                            name=f"i1S_{sfx}")
            nc.vector.tensor_scalar(i0S[:], tfS[:], 2.0, None, Alu.add)
            nc.vector.tensor_scalar(i1S[:], tfS[:], 70.0, None, Alu.add)
            idxR = pool.tile([128, 9, 2, NT, 8], i16, tag=f"idxR{sfx}",
                             name=f"idxR_{sfx}")
            for i, iS in ((0, i0S), (1, i1S)):
                out_ap = AP(idxR.tensor, idxR.offset + i * 128,
                            [[idxR.ap[0][0], 16], [1, 8], [8, NT], [256, 9]])
                nc.vector.tensor_copy(out_ap, iS[:])
            for cg in range(1, 8):
                nc.sync.dma_start(
                    idxR[16 * cg:16 * (cg + 1), :, :, :, :],
                    idxR[0:16, :, :, :, :])

            # re-zero the shared DVE accumulator for this half
            nc.vector.memset(accD[:], 0.0)
            # fp16 output staging for this half
            accH = pool.tile([128, NT, O], f16, tag=f"accH{sfx}",
                             name=f"accH_{sfx}")

            # psum accumulators for the PE-side combine (pos tiles 8..15)
            pa = [pacc.tile([128, 2, O], f32, tag=f"pa{j}",
                            name=f"pa_{sfx}_{j}")
                  for j in range(4)]

            # -------- per-tap: table, gather, combine --------
            qt_lo = 5 if hf == 0 else 0            # valid slab q-tiles
            qt_hi = 26 if hf == 0 else 21
            for k in range(K2):
                qts = list(range(qt_lo, qt_hi))
                pairs = [qts[j:j + 2] for j in range(0, len(qts), 2)]
                for pi, pr in enumerate(pairs):
                    ps = pst.tile([128, 2, O], f32, tag="tabps",
                                  name=f"tabps_{sfx}_{k}_{pi}")
                    for h, qt in enumerate(pr):
                        ir = r0 - 10 + 2 * qt      # image row of slab 2qt
                        for ct in range(2):
                            nc.tensor.matmul(
                                ps[:, h, :],
                                x64[:, ct, ir * W:(ir + 2) * W],
                                wtap[:, ct, k, :],
                                start=(ct == 0), stop=(ct == 1))
                    st = spool.tile([128, 2, O], f16, tag="tabst",
                                    name=f"tabst_{sfx}_{k}_{pi}")
                    nc.scalar.activation(st[:, 0:len(pr), :],
                                         ps[:, 0:len(pr), :], Act.Copy)
                    for h, qt in enumerate(pr):
                        # spread table-write DMAs over the HWDGE rings
                        weng = (nc.sync, nc.scalar)[(13 * k + pi) % 2]
                        weng.dma_start(
                            AP(tabs_d[hf][k], (2 * qt * 68 + 2) * O,
                               [[68 * O, 2], [O, 64], [1, O]]),
                            st[:, h, :])
                for i in range(2):
                    G = gpool.tile([128, NT, 512], f16, tag="G",
                                   name=f"G_{sfx}_{k}_{i}")
                    tab_ap = AP(tabs_d[hf][k], 0, [[O, TPIX - 1], [1, 512]])
                    # two half-gathers: idx<1024 covers pos tiles 0..7 (DVE
                    # combine half), idx>=1024 tiles 8..15 (PE half)
                    for hh in range(2):
                        nc.gpsimd.dma_gather(
                            G[:, 8 * hh:8 * (hh + 1), :], tab_ap,
                            idxR[:, k, i, 8 * hh:8 * (hh + 1), :],
                            num_idxs=P // 2, num_idxs_reg=nidx_reg,
                            elem_size=512, elem_step=O,
                            queue_num=(4 * k + 2 * i + hh) % 4,
                            single_packet=False)
                    wlo = w00 if i == 0 else w10
                    whi = w01 if i == 0 else w11
                    for pt in range(8):
                        nc.vector.scalar_tensor_tensor(
                            accD[:, pt, :], G[:, pt, 0:O],
                            wlo[:, pt, k:k + 1],
                            accD[:, pt, :], Alu.mult, Alu.add)
                        nc.vector.scalar_tensor_tensor(
                            accD[:, pt, :], G[:, pt, O:2 * O],
                            whi[:, pt, k:k + 1],
                            accD[:, pt, :], Alu.mult, Alu.add)
                    # pos tiles 8..15: scaled-identity matmuls accum in PSUM
                    for pt in range(8, NT):
                        for pix, wv in ((0, wlo), (1, whi)):
                            t = 4 * k + 2 * i + pix
                            dg = dpool.tile([128, 128], f16, tag="dg",
                                            name=f"dg_{sfx}_{k}_{i}_{pt}_{pix}")
                            if t % 3 == 0:
                                nc.vector.tensor_scalar(
                                    dg[:], idn[:], wv[:, pt, k:k + 1], None,
                                    Alu.mult)
                            else:
                                nc.scalar.activation(
                                    dg[:], idn[:], Act.Copy,
                                    scale=wv[:, pt, k:k + 1])
                            pb = pa[(pt - 8) // 2]
                            nc.tensor.matmul(
                                pb[:, (pt - 8) % 2, :], dg[:],
                                G[:, pt, pix * O:(pix + 1) * O],
                                start=(t == 0 and (pt - 8) % 2 == 0),
                                stop=(t == 35 and (pt - 8) % 2 == 1),
                                skip_group_check=True)

            # drain PE-side psum accumulators straight to fp16 staging
            for pt in range(8, NT):
                nc.scalar.activation(
                    accH[:, pt, :], pa[(pt - 8) // 2][:, (pt - 8) % 2, :],
                    Act.Copy)
            # DVE-side f32 accumulators -> fp16
            nc.scalar.activation(accH[:, 0:8, :], accD[:], Act.Copy)

            # -------- output --------
            base = hf * P * O
            nc.sync.dma_start(
                AP(out_d, base, [[O, 128], [128 * O, 8], [1, O]]),
                accH[:, 0:8, :])
            nc.sync.dma_start(
                AP(out_d, base + 8 * 128 * O, [[O, 128], [128 * O, 8], [1, O]]),
                accH[:, 8:NT, :])

    from concourse.library_overlay import lower_extended_insts
    lower_extended_insts(nc)
    if not os.environ.get("K_SIM"):
        _split_sync_waits(nc)
    return nc


def _split_sync_waits(nc, max_waits=1):
    """This walrus build encodes at most ~1 sem wait per instruction.
    Hoist extra waits onto preceding same-engine EventSemaphore ops."""
    import bass_rust
    import concourse.mybir as mybir
    for f in nc.m.functions:
        for bb in f.blocks:
            out = []
            changed = False
            for ins in bb.instructions:
                si = ins.sync_info
                if si is not None and len(si.on_wait) > max_waits \
                        and ins.engine is not None:
                    waits = list(si.on_wait)
                    extras, keep = waits[:-max_waits], waits[-max_waits:]
                    for j in range(0, len(extras), max_waits):
                        evs = mybir.InstNoOp(
                            name=f"nop_split_{nc.next_id()}", ins=[], outs=[],
                            engine=ins.engine)
                        evs.sync_info = bass_rust.SyncInfo(
                            on_wait=extras[j:j + max_waits], on_update=[])
                        out.append(evs)
                    ins.sync_info = bass_rust.SyncInfo(
                        on_wait=keep, on_update=list(si.on_update))
                    changed = True
                out.append(ins)
            if changed:
                bb.instructions = out


def _prep_host(inputs):
    """Build per-image x and the shared weight/grid arrays."""
    x = np.asarray(inputs["x"], np.float32)
    offset_w = np.asarray(inputs["offset_w"], np.float32)
    offset_b = np.asarray(inputs["offset_b"], np.float32)
    mod_w = np.asarray(inputs["mod_w"], np.float32)
    mod_b = np.asarray(inputs["mod_b"], np.float32)
    weight = np.asarray(inputs["weight"], np.float32)

    x64 = x.reshape(B, C, H * W)   # cast to fp16 per image at upload time

    wofs = np.concatenate([offset_w, mod_w], 0)            # [27, C, 3, 3]
    wofs = wofs.transpose(2, 3, 1, 0).reshape(9, C, 27)    # [tap, c, 27]
    wofs = np.ascontiguousarray(
        wofs.transpose(1, 0, 2).reshape(2, 128, 9, 27)).astype(np.float16)

    wtap = weight.reshape(O, C, 9).transpose(2, 1, 0)      # [tap, c, o]
    wtap = np.ascontiguousarray(
        wtap.transpose(1, 0, 2).reshape(2, 128, 9, O)).astype(np.float16)

    bias = np.zeros((1, 32), np.float32)
    bias[0, 0:18] = offset_b
    bias[0, 18:27] = mod_b

    return {
        "x64": x64,
        "wofs": wofs,
        "wtap": wtap,
        "bias": bias,
    }


def _get_runner():
    """Build the bass module once and wrap it in a cached jit fn."""
    if "runner" in _CACHE:
        return _CACHE["runner"]

    import sys
    if "/opt/trn_rl_repo" not in sys.path:
        sys.path.insert(0, "/opt/trn_rl_repo")
    import jax
    import concourse.mybir as mybir
    from concourse.bass2jax import (
        install_neuronx_cc_hook, _bass_exec_p, partition_id_tensor)

    nc = _build_module()
    install_neuronx_cc_hook()

    partition_name = (nc.partition_id_tensor.name
                      if nc.partition_id_tensor else None)
    in_names, out_names, out_avals = [], [], []
    for alloc in nc.m.functions[0].allocations:
        if not isinstance(alloc, mybir.MemoryLocationSet):
            continue
        name = alloc.memorylocations[0].name
        if alloc.kind == "ExternalInput":
            if name != partition_name:
                in_names.append(name)
        elif alloc.kind == "ExternalOutput":
            out_avals.append(jax.core.ShapedArray(
                tuple(alloc.tensor_shape), mybir.dt.np(alloc.dtype)))
            out_names.append(name)
    all_names = list(in_names)
    if partition_name is not None:
        all_names.append(partition_name)

    def _body(*args):
        # NOTE: no output-buffer operands — the PJRT runtime allocates NEFF
        # outputs itself and the kernel fully overwrites "out". The
        # neuronx_cc hook requires every operand to be a plain parameter.
        operands = list(args)
        if partition_name is not None:
            operands.append(partition_id_tensor())
        outs = _bass_exec_p.bind(
            *operands, out_avals=tuple(out_avals), in_names=tuple(all_names),
            out_names=tuple(out_names), lowering_input_output_aliases=(),
            sim_require_finite=True, sim_require_nnan=True, nc=nc)
        return tuple(outs)

    jit_fn = jax.jit(_body)
    _CACHE["runner"] = (jit_fn, in_names)
    return _CACHE["runner"]


def kernel(trace=False, **inputs):
    import jax
    from concurrent.futures import ThreadPoolExecutor
    jit_fn, in_names = _get_runner()
    feed = _prep_host(inputs)
    x32 = feed["x64"]
    # Two devices = two independent FIFO queues: device 1's uploads overlap
    # device 0's output downloads (the axon pipe is full-duplex but each
    # device executes its queue strictly in order). Weights/grids ship once
    # per device; per-image x is cast right before its async upload, and
    # each call is dispatched before the next upload is enqueued so image
    # b's exec isn't queued behind image b+1's transfer. Fetches run in
    # four threads so each output's download starts the moment its exec
    # finishes.
    devs = jax.devices()

    def submit(d_i):
        dev = devs[d_i]
        shared = {n: jax.device_put(feed[n], dev)
                  for n in in_names if n != "x64"}
        outs, first_x = [], None
        for b in (2 * d_i, 2 * d_i + 1):
            xb = jax.device_put(x32[b].astype(np.float16), dev)
            if first_x is None:
                first_x = xb
            args = [xb if n == "x64" else shared[n] for n in in_names]
            o = jit_fn(*args)[0]
            # start the output download server-side the moment its exec
            # finishes — the later np.asarray is then (nearly) free and no
            # per-fetch ~80ms client round trip lands on the critical path.
            o.copy_to_host_async()
            outs.append(o)
        return outs, first_x

    def run_once():
        outs = submit(0)[0] + submit(1)[0]
        out = np.empty((B, O, H, W), np.float32)
        for b in range(B):
            # restore each image as soon as its download lands; later
            # downloads stream in the background meanwhile.
            out[b] = np.asarray(outs[b]).reshape(2, ROWS, W, O).transpose(
                3, 0, 1, 2).reshape(O, H, W)
        return out

    try:
        return run_once()
    except Exception:
        # one defensive retry: the axon terminal occasionally reports a
        # transient device-unrecoverable on the first touch after an
        # earlier process died mid-run.
        import time
        time.sleep(2)
        return run_once()


# revision 28
# speedup vs baseline: 1.0603x; 1.0232x over previous
"""Trainium2 Bass kernel for modulated deformable conv v2 (DCNv2).

Problem (hardcoded): x [4,256,64,64] f32; offset_w [18,256,3,3]; offset_b [18];
mod_w [9,256,3,3]; mod_b [9]; weight [256,256,3,3] -> out [4,256,64,64] f32.

End-to-end latency is dominated by the ~43MB/s full-duplex axon pipe, so the
wire format is minimal and pipelined: the device module processes ONE full
image (both 32-row halves looped inside); fp16 weights (~1.3MB) ship once
per device, then 4 async jit calls across 2 devices stream one fp16 image
each (2.1MB); output downloads (fp16, 2.1MB/image) are enqueued eagerly via
copy_to_host_async and overlap later uploads through the duplex pipe. The
PJRT executable is cached across calls; identity matrices and the
structural sampling grids are NEFF consts (only a 128B bias row ships).

Device algorithm per image, per half (r0 = 32*half):
  1. offset/mod conv (27 out ch) as accumulating matmuls with weights
     stationary (rhs = padded-x rows built on device from the image).
  2. index/weight math in [pos-partition, free] layout:
     py/px -> floor via round-to-nearest magic -> bilinear*2*sigmoid weights
     w00..w11 [128,NT,9] and int16 pixel indices into a 52x68(+2 guard)
     zero-ring padded table in half-slab coords (image rows r0-10..r0+41);
     indices in the gather's 16-row wrapped layout. The slab-relative grid
     is identical for both halves.
  3. per tap k: table y_k^T = x^T @ W_k^T on PE (fp16) for the in-image
     rows only; out-of-image rows and ring pads zero-DMA'd.
  4. per tap: 2 dma_gathers (rows y0, y0+1), payload = 2 adjacent pixels
     (512 fp16 = 1KB), landing [128 pos, NT, 512].
  5. combine: pos tiles 0..7 on DVE via scalar_tensor_tensor into f32 SBUF;
     pos tiles 8..15 on PE via scaled-identity diagonal matmuls accumulating
     in 4 PSUM banks; both drained to fp16 staging.
  6. DMA out [2048 pos, 256 o] fp16 per half; host restores NCHW f32.
"""

import numpy as np

B, C, H, W = 4, 256, 64, 64
O, K2 = 256, 9
ROWS = 32                  # output rows per half
P = ROWS * W               # positions per half = 2048
NT = P // 128              # position tiles per half = 16
TPW = 68                   # table row width in pixels
TROWS = 52                 # table rows: image rows r0-10 .. r0+41
TPIX = TROWS * TPW + 2     # +2 guard pixels = 3538
XR = ROWS + 2              # padded-x rows per half = 34
TQT = TROWS // 2           # table q-tiles (2 rows each) = 26

_CACHE = {}


def _patch_tile_drain():
    """This walrus build's TPB_CTRL encodes at most ~1 sem wait; Tile's
    kernel-tail drain aggregates the whole global clock onto one Drain.
    Spread the waits across a chain of single-wait drains instead."""
    import bass_rust
    from concourse.tile import TileContext, ScopedClock

    if getattr(TileContext, "_drain_patched", False):
        return

    def _drain_and_barrier(self, tick_clock, wait_clock):
        import os
        nc = self.nc
        drain_inst = nc.sync.drain()
        wait_clock.add_sem_waits(
            drain_inst.ins, ScopedClock({None: tick_clock.global_clock}))
        si = drain_inst.ins.sync_info
        if not os.environ.get("K_SIM") and si is not None \
                and len(si.on_wait) > 1:
            waits = list(si.on_wait)
            ups = list(si.on_update)
            drain_inst.ins.sync_info = bass_rust.SyncInfo(
                on_wait=waits[:1], on_update=ups)
            for j in range(1, len(waits)):
                extra = nc.sync.drain()
                extra.ins.sync_info = bass_rust.SyncInfo(
                    on_wait=[waits[j]], on_update=[])
        nc.all_engine_barrier()
        assert self.sems is not None
        popped = nc._tile_sem_poison_stack.pop()
        assert popped is self._sem_poison
        nc.clear_and_free_semaphores(list(self.sems.allocated().values()))
        nc.all_engine_barrier()

    TileContext._drain_and_barrier = _drain_and_barrier
    TileContext._drain_patched = True


def _build_module():
    import os
    import concourse.bass as bass
    import concourse.mybir as mybir
    import concourse.tile as tile
    from concourse.library_config import mlp as mlp_lib
    from contextlib import ExitStack

    _patch_tile_drain()

    dt = mybir.dt
    f32, f16, i16 = dt.float32, dt.float16, dt.int16
    Alu = mybir.AluOpType
    Act = mybir.ActivationFunctionType
    AP = bass.AP

    nc = bass.Bass(num_swdge_queues=4)

    x64_d = nc.dram_tensor("x64", [C, H * W], f16, kind="ExternalInput")
    wofs_d = nc.dram_tensor("wofs", [2, 128, 9, 27], f16, kind="ExternalInput")
    wtap_d = nc.dram_tensor("wtap", [2, 128, 9, O], f16, kind="ExternalInput")
    # bias[0, 0:18] = offset_b, bias[0, 18:27] = mod_b
    bias_d = nc.dram_tensor("bias", [1, 32], f32, kind="ExternalInput")
    id27_d = nc.inline_tensor(np.eye(27, dtype=np.float32), "id27")
    idn_d = nc.inline_tensor(np.eye(128, dtype=np.float16), "idn")
    # structural sampling grids in slab coords (identical for both halves,
    # bias added on device): bgyc[p,t,k] = row(p,t) + ky - 1 - 0.49999,
    # bgxc[p,t,k] = col(p) + kx - 1 - 0.49999 (magic-floor epsilon folded).
    _p = np.arange(P)
    _gy = np.empty((128, NT, 9), np.float32)
    _gx = np.empty((128, NT, 9), np.float32)
    for _k in range(9):
        _ky, _kx = divmod(_k, 3)
        _gy[:, :, _k] = (_p // 64 + 10 + _ky - 1 - 0.49999).reshape(NT, 128).T
        _gx[:, :, _k] = (_p % 64 + _kx - 1 - 0.49999).reshape(NT, 128).T
    bgyc_d = nc.inline_tensor(_gy.reshape(128, NT * 9), "bgyc")
    bgxc_d = nc.inline_tensor(_gx.reshape(128, NT * 9), "bgxc")
    out_d = nc.dram_tensor("out", [2 * P, O], f16, kind="ExternalOutput")

    # per-(half, tap) gather tables, double-buffered across halves
    tabs_d = [[nc.dram_tensor(f"tab{h}_{k}", [TPIX, O], f16)
               for k in range(K2)] for h in range(2)]

    with tile.TileContext(nc) as tc, ExitStack() as ctx:
        pool = ctx.enter_context(tc.tile_pool(name="main", bufs=1))
        psc = ctx.enter_context(tc.tile_pool(name="psc", bufs=1, space="PSUM"))
        pst = ctx.enter_context(tc.tile_pool(name="pst", bufs=3, space="PSUM"))
        pacc = ctx.enter_context(tc.tile_pool(name="pacc", bufs=1, space="PSUM"))
        dpool = ctx.enter_context(tc.tile_pool(name="diag", bufs=8))
        gpool = ctx.enter_context(tc.tile_pool(name="gath", bufs=2))
        spool = ctx.enter_context(tc.tile_pool(name="stage", bufs=6))

        # ---------------- load inputs ----------------
        nc.gpsimd.load_library(mlp_lib)
        x64 = pool.tile([128, 2, H * W], f16, tag="x64", name="x64_sb")
        nc.sync.dma_start(
            x64[:],
            AP(x64_d, 0, [[H * W, 128], [128 * H * W, 2], [1, H * W]]))
        wofs = pool.tile([128, 2, 9, 27], f16, tag="wofs", name="wofs_sb")
        nc.sync.dma_start(
            wofs[:],
            AP(wofs_d, 0, [[9 * 27, 128], [128 * 9 * 27, 2], [1, 9 * 27]]))
        wtap = pool.tile([128, 2, 9, O], f16, tag="wtap", name="wtap_sb")
        nc.sync.dma_start(
            wtap[:],
            AP(wtap_d, 0, [[9 * O, 128], [128 * 9 * O, 2], [1, 9 * O]]))
        bgy = pool.tile([128, NT, 9], f32, tag="bgy", name="bgy_sb")
        nc.sync.dma_start(bgy[:], bgyc_d[:, :])
        bgx = pool.tile([128, NT, 9], f32, tag="bgx", name="bgx_sb")
        nc.sync.dma_start(bgx[:], bgxc_d[:, :])
        # replicate the 32-float bias row across partitions (log doubling),
        # then fold the per-tap biases into the structural grids.
        bias = pool.tile([128, 32], f32, tag="bias", name="bias_sb")
        nc.sync.dma_start(bias[0:1, :], bias_d[:, :])
        for _d in range(7):
            w_ = 1 << _d
            nc.sync.dma_start(bias[w_:2 * w_, :], bias[0:w_, :])
        bias_y = AP(bias.tensor, bias.offset, [bias.ap[0], [0, NT], [2, 9]])
        bias_x = AP(bias.tensor, bias.offset + 1,
                    [bias.ap[0], [0, NT], [2, 9]])
        bias_m = AP(bias.tensor, bias.offset + 18,
                    [bias.ap[0], [0, NT], [1, 9]])
        nc.vector.tensor_tensor(bgy[:], bgy[:], bias_y, Alu.add)
        nc.vector.tensor_tensor(bgx[:], bgx[:], bias_x, Alu.add)
        id27 = pool.tile([27, 27], f32, tag="id27", name="id27_sb")
        nc.sync.dma_start(id27[:], id27_d[:, :])
        idn = pool.tile([128, 128], f16, tag="idn", name="idn_sb")
        nc.sync.dma_start(idn[:], idn_d[:, :])

        # zero tile for table zeroing (big band DMAs use all 1360 cols)
        zt = pool.tile([128, 1360], f16, tag="zt", name="zt_sb")
        nc.gpsimd.memset(zt[:], 0.0)

        # DVE-side combine accumulator (shared across halves, re-zeroed)
        accD = pool.tile([128, 8, O], f32, tag="accD", name="accD_sb")

        # one shared GPSIMD register for every gather's index count (72
        # per-call to_reg allocations would exhaust the register file)
        nidx_reg = nc.gpsimd.to_reg(P // 2)

        # ---------------- per-half pipeline ----------------
        for hf in range(2):
            r0 = ROWS * hf
            sfx = f"h{hf}"

            # -------- padded conv input [128c, 2ct, 34r x 66] --------
            # image rows r0-1 .. r0+32 into 66-wide zeroed rows at col 1.
            xpad = pool.tile([128, 2, XR * 66], f16, tag=f"xpad{sfx}",
                             name=f"xpad_{sfx}")
            nc.vector.memset(xpad[:], 0.0)
            a0, a1 = max(0, r0 - 1), min(H, r0 + XR - 1)
            for ct in range(2):
                xp_ct = xpad[:, ct, :]
                x6_ct = x64[:, ct, :]
                nc.sync.dma_start(
                    AP(xp_ct.tensor,
                       xp_ct.offset + (a0 - (r0 - 1)) * 66 + 1,
                       [xp_ct.ap[0], [66, a1 - a0], [1, W]]),
                    AP(x6_ct.tensor, x6_ct.offset + a0 * W,
                       [x6_ct.ap[0], [W, a1 - a0], [1, W]]))

            # -------- gather-table zeroing --------
            # valid slab rows: those with image rows r0-10+2qt in [0, 64);
            # h0: slab rows 10..51 valid (zero band rows 0..9);
            # h1: slab rows 0..41 valid (zero band rows 42..51).
            for k in range(K2):
                t = tabs_d[hf][k]
                if hf == 0:
                    # zero band px [0, 680) incl. its col pads
                    nc.scalar.dma_start(
                        AP(t, 0, [[1360, 128], [1, 1360]]), zt[:])
                    # col-pad runs (r,66),(r,67),(r+1,0),(r+1,1), r=10..50
                    nc.scalar.dma_start(
                        AP(t, (10 * 68 + 66) * O, [[68 * O, 41], [1, 4 * O]]),
                        zt[0:41, 0:1024])
                    # last-row right pads px 3534,3535
                    nc.sync.dma_start(
                        AP(t, (51 * 68 + 66) * O, [[1, 2 * O]]),
                        zt[0:1, 0:512])
                    # first-valid-row left pads px 680,681
                    nc.sync.dma_start(
                        AP(t, (10 * 68) * O, [[1, 2 * O]]), zt[0:1, 0:512])
                else:
                    # zero band px [2856, 3536)
                    nc.scalar.dma_start(
                        AP(t, 2856 * O, [[1360, 128], [1, 1360]]), zt[:])
                    # col-pad runs r=0..40
                    nc.scalar.dma_start(
                        AP(t, 66 * O, [[68 * O, 41], [1, 4 * O]]),
                        zt[0:41, 0:1024])
                    # row-0 left pads px 0,1
                    nc.sync.dma_start(AP(t, 0, [[1, 2 * O]]), zt[0:1, 0:512])
                    # last-valid-row right pads px 2854,2855
                    nc.sync.dma_start(
                        AP(t, (41 * 68 + 66) * O, [[1, 2 * O]]),
                        zt[0:1, 0:512])
                # guard px 3536,3537
                nc.sync.dma_start(
                    AP(t, (TROWS * 68) * O, [[1, 2 * O]]), zt[0:1, 0:512])

            # -------- offset/mod conv --------
            conv_sb = pool.tile([27, P], f32, tag=f"conv{sfx}",
                                name=f"conv_sb_{sfx}")
            for pc in range(4):
                ps = psc.tile([27, 512], f32, tag="convps",
                              name=f"convps_{sfx}_{pc}")
                n = 0
                for ct in range(2):
                    xp_ct = xpad[:, ct, :]
                    for tap in range(9):
                        dy, dx = divmod(tap, 3)
                        rhs = AP(xp_ct.tensor,
                                 xp_ct.offset + (8 * pc + dy) * 66 + dx,
                                 [xp_ct.ap[0], [66, 8], [1, 64]])
                        nc.tensor.matmul(
                            ps[:], wofs[:, ct, tap, :], rhs,
                            start=(n == 0), stop=(n == 17))
                        n += 1
                nc.scalar.activation(conv_sb[:, 512 * pc:512 * (pc + 1)],
                                     ps[:], Act.Copy)
            ofs = pool.tile([128, NT, 27], f32, tag=f"ofs{sfx}",
                            name=f"ofs_{sfx}")
            for pt in range(NT):
                ps2 = psc.tile([128, 27], f32, tag="convps",
                               name=f"trps_{sfx}_{pt}")
                nc.tensor.transpose(
                    ps2[:], conv_sb[:, 128 * pt:128 * (pt + 1)], id27[:])
                nc.scalar.activation(ofs[:, pt, :], ps2[:], Act.Copy)

            # -------- index/weight math --------
            def t144(nm):
                return pool.tile([128, NT, 9], f32, tag=f"{nm}{sfx}",
                                 name=f"{nm}_{sfx}")

            offy = AP(ofs.tensor, ofs.offset, [ofs.ap[0], [27, NT], [2, 9]])
            offx = AP(ofs.tensor, ofs.offset + 1,
                      [ofs.ap[0], [27, NT], [2, 9]])
            offm = AP(ofs.tensor, ofs.offset + 18,
                      [ofs.ap[0], [27, NT], [1, 9]])

            py, px = t144("py"), t144("px")
            nc.vector.tensor_tensor(py[:], offy, bgy[:], Alu.add)
            nc.vector.tensor_tensor(px[:], offx, bgx[:], Alu.add)

            # floor via round-to-nearest magic number: the host grids carry
            # -0.49999 so py here is py_true - 0.49999 and y0 = RN(py+M) - M
            # equals floor(py_true) (up to an O(1e-4) edge band, harmless).
            MAGIC = 12582912.0  # 1.5 * 2**23
            EPS = 0.49999
            fy, fx = t144("fy"), t144("fx")
            y0, x0 = t144("y0"), t144("x0")
            nc.vector.tensor_scalar(y0[:], py[:], MAGIC, -MAGIC,
                                    Alu.add, Alu.add)
            nc.vector.tensor_scalar(x0[:], px[:], MAGIC, -MAGIC,
                                    Alu.add, Alu.add)
            nc.vector.scalar_tensor_tensor(
                fy[:], py[:], EPS, y0[:], Alu.add, Alu.subtract)
            nc.vector.scalar_tensor_tensor(
                fx[:], px[:], EPS, x0[:], Alu.add, Alu.subtract)
            nc.vector.tensor_scalar(y0[:], y0[:], 0.0, float(TROWS - 2),
                                    Alu.max, Alu.min)
            nc.vector.tensor_scalar(x0[:], x0[:], -2.0, 64.0,
                                    Alu.max, Alu.min)

            # mask2 = 2*sigmoid(mod + mod_b); factor 2 folded into gy2/fy2
            m2 = t144("m2")
            nc.vector.tensor_tensor(m2[:], offm, bias_m, Alu.add)
            nc.scalar.activation(m2[:], m2[:], Act.Sigmoid)
            gy2, fy2 = t144("gy2"), t144("fy2")
            nc.vector.tensor_scalar(gy2[:], fy[:], -2.0, 2.0,
                                    Alu.mult, Alu.add)
            nc.vector.tensor_scalar(fy2[:], fy[:], 2.0, None, Alu.mult)
            gx1 = t144("gx1")
            nc.vector.tensor_scalar(gx1[:], fx[:], -1.0, 1.0,
                                    Alu.mult, Alu.add)
            wa, wb = t144("wa"), t144("wb")
            nc.vector.tensor_tensor(wa[:], gy2[:], m2[:], Alu.mult)
            nc.vector.tensor_tensor(wb[:], fy2[:], m2[:], Alu.mult)
            w00, w01 = t144("w00"), t144("w01")
            w10, w11 = t144("w10"), t144("w11")
            nc.vector.tensor_tensor(w00[:], wa[:], gx1[:], Alu.mult)
            nc.vector.tensor_tensor(w01[:], wa[:], fx[:], Alu.mult)
            nc.vector.tensor_tensor(w10[:], wb[:], gx1[:], Alu.mult)
            nc.vector.tensor_tensor(w11[:], wb[:], fx[:], Alu.mult)

            # indices, computed directly in the gather's wrapped layout:
            # partition r holds positions p = 16g + r; free = (k, i, t, g).
            ycS = pool.tile([16, 8, NT, 9], f32, tag=f"ycS{sfx}",
                            name=f"ycS_{sfx}")
            xcS = pool.tile([16, 8, NT, 9], f32, tag=f"xcS{sfx}",
                            name=f"xcS_{sfx}")
            for g in range(8):
                nc.sync.dma_start(ycS[0:16, g, :, :],
                                  y0[16 * g:16 * (g + 1), :, :])
                nc.sync.dma_start(xcS[0:16, g, :, :],
                                  x0[16 * g:16 * (g + 1), :, :])
            tfS = pool.tile([16, 8, NT, 9], f32, tag=f"tfS{sfx}",
                            name=f"tfS_{sfx}")
            nc.vector.scalar_tensor_tensor(
                tfS[:], ycS[:], 68.0, xcS[:], Alu.mult, Alu.add)
            i0S = pool.tile([16, 8, NT, 9], f32, tag=f"i0S{sfx}",
                            name=f"i0S_{sfx}")
            i1S = pool.tile([16, 8, NT, 9], f32, tag=f"i1S{sfx}",
                            name=f"i1S_{sfx}")
            nc.vector.tensor_scalar(i0S[:], tfS[:], 2.0, None, Alu.add)
            nc.vector.tensor_scalar(i1S[:], tfS[:], 70.0, None, Alu.add)
            idxR = pool.tile([128, 9, 2, NT, 8], i16, tag=f"idxR{sfx}",
                             name=f"idxR_{sfx}")
            for i, iS in ((0, i0S), (1, i1S)):
                out_ap = AP(idxR.tensor, idxR.offset + i * 128,
                            [[idxR.ap[0][0], 16], [1, 8], [8, NT], [256, 9]])
                nc.vector.tensor_copy(out_ap, iS[:])
            for cg in range(1, 8):
                nc.sync.dma_start(
                    idxR[16 * cg:16 * (cg + 1), :, :, :, :],
                    idxR[0:16, :, :, :, :])

            # re-zero the shared DVE accumulator for this half
            nc.vector.memset(accD[:], 0.0)
            # fp16 output staging for this half
            accH = pool.tile([128, NT, O], f16, tag=f"accH{sfx}",
                             name=f"accH_{sfx}")

            # psum accumulators for the PE-side combine (pos tiles 8..15)
            pa = [pacc.tile([128, 2, O], f32, tag=f"pa{j}",
                            name=f"pa_{sfx}_{j}")
                  for j in range(4)]

            # -------- per-tap: table, gather, combine --------
            qt_lo = 5 if hf == 0 else 0            # valid slab q-tiles
            qt_hi = 26 if hf == 0 else 21
            for k in range(K2):
                qts = list(range(qt_lo, qt_hi))
                pairs = [qts[j:j + 2] for j in range(0, len(qts), 2)]
                for pi, pr in enumerate(pairs):
                    ps = pst.tile([128, 2, O], f32, tag="tabps",
                                  name=f"tabps_{sfx}_{k}_{pi}")
                    for h, qt in enumerate(pr):
                        ir = r0 - 10 + 2 * qt      # image row of slab 2qt
                        for ct in range(2):
                            nc.tensor.matmul(
                                ps[:, h, :],
                                x64[:, ct, ir * W:(ir + 2) * W],
                                wtap[:, ct, k, :],
                                start=(ct == 0), stop=(ct == 1))
                    st = spool.tile([128, 2, O], f16, tag="tabst",
                                    name=f"tabst_{sfx}_{k}_{pi}")
                    nc.scalar.activation(st[:, 0:len(pr), :],
                                         ps[:, 0:len(pr), :], Act.Copy)
                    for h, qt in enumerate(pr):
                        # spread table-write DMAs over the HWDGE rings
                        weng = (nc.sync, nc.scalar)[(13 * k + pi) % 2]
                        weng.dma_start(
                            AP(tabs_d[hf][k], (2 * qt * 68 + 2) * O,
                               [[68 * O, 2], [O, 64], [1, O]]),
                            st[:, h, :])
                for i in range(2):
                    G = gpool.tile([128, NT, 512], f16, tag="G",
                                   name=f"G_{sfx}_{k}_{i}")
                    tab_ap = AP(tabs_d[hf][k], 0, [[O, TPIX - 1], [1, 512]])
                    # two half-gathers: idx<1024 covers pos tiles 0..7 (DVE
                    # combine half), idx>=1024 tiles 8..15 (PE half)
                    for hh in range(2):
                        nc.gpsimd.dma_gather(
                            G[:, 8 * hh:8 * (hh + 1), :], tab_ap,
                            idxR[:, k, i, 8 * hh:8 * (hh + 1), :],
                            num_idxs=P // 2, num_idxs_reg=nidx_reg,
                            elem_size=512, elem_step=O,
                            queue_num=(4 * k + 2 * i + hh) % 4,
                            single_packet=False)
                    wlo = w00 if i == 0 else w10
                    whi = w01 if i == 0 else w11
                    for pt in range(8):
                        nc.vector.scalar_tensor_tensor(
                            accD[:, pt, :], G[:, pt, 0:O],
                            wlo[:, pt, k:k + 1],
                            accD[:, pt, :], Alu.mult, Alu.add)
                        nc.vector.scalar_tensor_tensor(
                            accD[:, pt, :], G[:, pt, O:2 * O],
                            whi[:, pt, k:k + 1],
                            accD[:, pt, :], Alu.mult, Alu.add)
                    # pos tiles 8..15: scaled-identity matmuls accum in PSUM
                    for pt in range(8, NT):
                        for pix, wv in ((0, wlo), (1, whi)):
                            t = 4 * k + 2 * i + pix
                            dg = dpool.tile([128, 128], f16, tag="dg",
                                            name=f"dg_{sfx}_{k}_{i}_{pt}_{pix}")
                            if t % 3 == 0:
                                nc.vector.tensor_scalar(
                                    dg[:], idn[:], wv[:, pt, k:k + 1], None,
                                    Alu.mult)
                            else:
                                nc.scalar.activation(
                                    dg[:], idn[:], Act.Copy,
                                    scale=wv[:, pt, k:k + 1])
                            pb = pa[(pt - 8) // 2]
                            nc.tensor.matmul(
                                pb[:, (pt - 8) % 2, :], dg[:],
                                G[:, pt, pix * O:(pix + 1) * O],
                                start=(t == 0 and (pt - 8) % 2 == 0),
                                stop=(t == 35 and (pt - 8) % 2 == 1),
                                skip_group_check=True)

            # drain PE-side psum accumulators straight to fp16 staging
            for pt in range(8, NT):
                nc.scalar.activation(
                    accH[:, pt, :], pa[(pt - 8) // 2][:, (pt - 8) % 2, :],
                    Act.Copy)
            # DVE-side f32 accumulators -> fp16
            nc.scalar.activation(accH[:, 0:8, :], accD[:], Act.Copy)

            # -------- output --------
            base = hf * P * O
            nc.sync.dma_start(
                AP(out_d, base, [[O, 128], [128 * O, 8], [1, O]]),
                accH[:, 0:8, :])
            nc.sync.dma_start(
                AP(out_d, base + 8 * 128 * O, [[O, 128], [128 * O, 8], [1, O]]),
                accH[:, 8:NT, :])

    from concourse.library_overlay import lower_extended_insts
    lower_extended_insts(nc)
    if not os.environ.get("K_SIM"):
        _split_sync_waits(nc)
    return nc


def _split_sync_waits(nc, max_waits=1):
    """This walrus build encodes at most ~1 sem wait per instruction.
    Hoist extra waits onto preceding same-engine EventSemaphore ops."""
    import bass_rust
    import concourse.mybir as mybir
    for f in nc.m.functions:
        for bb in f.blocks:
            out = []
            changed = False
            for ins in bb.instructions:
                si = ins.sync_info
                if si is not None and len(si.on_wait) > max_waits \
                        and ins.engine is not None:
                    waits = list(si.on_wait)
                    extras, keep = waits[:-max_waits], waits[-max_waits:]
                    for j in range(0, len(extras), max_waits):
                        evs = mybir.InstNoOp(
                            name=f"nop_split_{nc.next_id()}", ins=[], outs=[],
                            engine=ins.engine)
                        evs.sync_info = bass_rust.SyncInfo(
                            on_wait=extras[j:j + max_waits], on_update=[])
                        out.append(evs)
                    ins.sync_info = bass_rust.SyncInfo(
                        on_wait=keep, on_update=list(si.on_update))
                    changed = True
                out.append(ins)
            if changed:
                bb.instructions = out


def _prep_host(inputs):
    """Build per-image x and the shared weight/grid arrays."""
    x = np.asarray(inputs["x"], np.float32)
    offset_w = np.asarray(inputs["offset_w"], np.float32)
    offset_b = np.asarray(inputs["offset_b"], np.float32)
    mod_w = np.asarray(inputs["mod_w"], np.float32)
    mod_b = np.asarray(inputs["mod_b"], np.float32)
    weight = np.asarray(inputs["weight"], np.float32)

    x64 = x.reshape(B, C, H * W)   # cast to fp16 per image at upload time

    wofs = np.concatenate([offset_w, mod_w], 0)            # [27, C, 3, 3]
    wofs = wofs.transpose(2, 3, 1, 0).reshape(9, C, 27)    # [tap, c, 27]
    wofs = np.ascontiguousarray(
        wofs.transpose(1, 0, 2).reshape(2, 128, 9, 27)).astype(np.float16)

    wtap = weight.reshape(O, C, 9).transpose(2, 1, 0)      # [tap, c, o]
    wtap = np.ascontiguousarray(
        wtap.transpose(1, 0, 2).reshape(2, 128, 9, O)).astype(np.float16)

    bias = np.zeros((1, 32), np.float32)
    bias[0, 0:18] = offset_b
    bias[0, 18:27] = mod_b

    return {
        "x64": x64,
        "wofs": wofs,
        "wtap": wtap,
        "bias": bias,
    }


def _get_runner():
    """Build the bass module once and wrap it in a cached jit fn."""
    if "runner" in _CACHE:
        return _CACHE["runner"]

    import sys
    if "/opt/trn_rl_repo" not in sys.path:
        sys.path.insert(0, "/opt/trn_rl_repo")
    import jax
    import concourse.mybir as mybir
    from concourse.bass2jax import (
        install_neuronx_cc_hook, _bass_exec_p, partition_id_tensor)

    nc = _build_module()
    install_neuronx_cc_hook()

    partition_name = (nc.partition_id_tensor.name
                      if nc.partition_id_tensor else None)
    in_names, out_names, out_avals = [], [], []
    for alloc in nc.m.functions[0].allocations:
        if not isinstance(alloc, mybir.MemoryLocationSet):
            continue
        name = alloc.memorylocations[0].name
        if alloc.kind == "ExternalInput":
            if name != partition_name:
                in_names.append(name)
        elif alloc.kind == "ExternalOutput":
            out_avals.append(jax.core.ShapedArray(
                tuple(alloc.tensor_shape), mybir.dt.np(alloc.dtype)))
            out_names.append(name)
    all_names = list(in_names)
    if partition_name is not None:
        all_names.append(partition_name)

    def _body(*args):
        # NOTE: no output-buffer operands — the PJRT runtime allocates NEFF
        # outputs itself and the kernel fully overwrites "out". The
        # neuronx_cc hook requires every operand to be a plain parameter.
        operands = list(args)
        if partition_name is not None:
            operands.append(partition_id_tensor())
        outs = _bass_exec_p.bind(
            *operands, out_avals=tuple(out_avals), in_names=tuple(all_names),
            out_names=tuple(out_names), lowering_input_output_aliases=(),
            sim_require_finite=True, sim_require_nnan=True, nc=nc)
        return tuple(outs)

    jit_fn = jax.jit(_body)
    _CACHE["runner"] = (jit_fn, in_names)
    return _CACHE["runner"]


def kernel(trace=False, **inputs):
    import jax
    jit_fn, in_names = _get_runner()
    feed = _prep_host(inputs)
    x32 = feed["x64"]
    # Two devices = two independent FIFO queues: each device executes its
    # queue strictly in order, but the axon pipe is full-duplex, so one
    # device's output downloads overlap the other's uploads. Weights ship
    # once per device; per-image x is cast right before its async upload,
    # and each call is dispatched (and its download enqueued) before the
    # next upload so nothing queues behind a later transfer.
    devs = jax.devices()

    def submit(d_i):
        dev = devs[d_i]
        shared = {n: jax.device_put(feed[n], dev)
                  for n in in_names if n != "x64"}
        outs = []
        for b in (2 * d_i, 2 * d_i + 1):
            xb = jax.device_put(x32[b].astype(np.float16), dev)
            args = [xb if n == "x64" else shared[n] for n in in_names]
            o = jit_fn(*args)[0]
            # start the output download server-side the moment its exec
            # finishes — the later np.asarray is then (nearly) free and no
            # per-fetch ~80ms client round trip lands on the critical path.
            o.copy_to_host_async()
            outs.append(o)
        return outs

    def run_once():
        outs = submit(0) + submit(1)
        out = np.empty((B, O, H, W), np.float32)
        for b in range(B):
            # restore each image as soon as its download lands; later
            # downloads stream in the background meanwhile.
            out[b] = np.asarray(outs[b]).reshape(2, ROWS, W, O).transpose(
                3, 0, 1, 2).reshape(O, H, W)
        return out

    try:
        return run_once()
    except Exception:
        # one defensive retry: the axon terminal occasionally reports a
        # transient device-unrecoverable on the first touch after an
        # earlier process died mid-run.
        import time
        time.sleep(2)
        return run_once()
